# revision 20
# baseline (speedup 1.0000x reference)
"""Trainium2 Bass kernel for nn_Attention_1486058684451.

Decode-style attention with sliding-window KV-cache update, sharded
tensor-parallel over 8 NeuronCores: each core owns 4 query heads + 1 KV
head (wq/wk/wv output rows, wo input columns, cache KV-head slice).
ReduceScatter combines the wo partials; the host concatenates shards.

All shapes hardcoded per the problem spec:
  x[8,16,4096], mask[16,4096], freqs_cis[16,64,2],
  cache_k/v[8,4096,8,128], wq[4096,4096], wk/wv[1024,4096], wo[4096,4096]
"""

import math
from contextlib import ExitStack

import numpy as np

B, S, D = 8, 16, 4096
NH, NKV, HD = 32, 8, 128
L = 4096
NCORES = 8
T = B * S                   # 128 tokens
HQ = NH // NCORES           # 4 local q heads
QW = HQ * HD                # 512 local q-projection width
NDC = D // 128              # 32 contraction chunks
NLC = L // 128              # 32 cache chunks
GRP = 8                     # score chunks packed per PSUM bank
HS = HQ * S                 # 64 score columns per (head, seq)

BF16_ATTN = True            # scores + PV matmuls in bf16 (fp32 accumulate)
BF16_PROJ = True            # q/k/v/o projections in bf16 (weights host-cast)

_CACHE = {}


def _build(use_mask):
    import concourse.tile as tile
    from concourse import bacc, bass_isa, mybir
    from concourse.masks import make_identity

    f32 = mybir.dt.float32
    bf16 = mybir.dt.bfloat16
    adt = bf16 if BF16_ATTN else f32     # attention operand dtype
    pdt = bf16 if BF16_PROJ else f32     # projection operand dtype

    nc = bacc.Bacc(
        "TRN2",
        target_bir_lowering=False,
        debug=False,
        num_devices=NCORES,
    )

    # ---- kernel I/O (per-core shards, host-prepared) ----
    xT = nc.dram_tensor("xT", [D, T], pdt, kind="ExternalInput")
    wqT = nc.dram_tensor("wqT", [D, QW], pdt, kind="ExternalInput")
    wkT = nc.dram_tensor("wkT", [D, HD], pdt, kind="ExternalInput")
    wvT = nc.dram_tensor("wvT", [D, HD], pdt, kind="ExternalInput")
    woT = nc.dram_tensor("woT", [QW, D], pdt, kind="ExternalInput")
    ck = nc.dram_tensor("ck", [B, L, HD], f32, kind="ExternalInput")
    cv = nc.dram_tensor("cv", [B, L, HD], f32, kind="ExternalInput")
    cosq = nc.dram_tensor("cosq", [T, QW], f32, kind="ExternalInput")
    sinq = nc.dram_tensor("sinq", [T, QW], f32, kind="ExternalInput")
    cosk = nc.dram_tensor("cosk", [T, HD], f32, kind="ExternalInput")
    sink = nc.dram_tensor("sink", [T, HD], f32, kind="ExternalInput")
    maskT = (
        nc.dram_tensor("maskT", [L, HS], f32, kind="ExternalInput")
        if use_mask else None
    )

    y = nc.dram_tensor("y", [S, D], f32, kind="ExternalOutput")
    cko = nc.dram_tensor("cko", [B, L, HD], f32, kind="ExternalOutput")
    cvo = nc.dram_tensor("cvo", [B, L, HD], f32, kind="ExternalOutput")

    NRS = 2
    DH = D // NRS
    rs_in = [nc.dram_tensor(f"rs_in{j}", [T, DH], f32) for j in range(NRS)]
    rs_out = [nc.dram_tensor(f"rs_out{j}", [S, DH], f32) for j in range(NRS)]

    with tile.TileContext(nc) as tc, ExitStack() as top:
        persist = top.enter_context(tc.tile_pool(name="persist", bufs=1))
        ptr = top.enter_context(tc.tile_pool(name="ptr", bufs=2, space="PSUM"))

        ident = persist.tile([128, 128], f32, tag="ident")
        make_identity(nc, ident[:])
        ident_a = persist.tile([128, 128], adt, tag="ident_a")
        nc.vector.tensor_copy(ident_a[:], ident[:])
        ones_row = persist.tile([1, 128], f32, tag="ones_row")
        nc.gpsimd.memset(ones_row[:], 1.0)

        xT_s = persist.tile([128, NDC * T], pdt, tag="xT_s")
        nc.sync.dma_start(
            xT_s[:].rearrange("p (c t) -> p c t", c=NDC),
            xT[:, :].rearrange("(c p) t -> p c t", p=128),
        )
        cosq_s = persist.tile([T, QW], f32, tag="cosq_s")
        nc.sync.dma_start(cosq_s[:], cosq[:, :])
        sinq_s = persist.tile([T, QW], f32, tag="sinq_s")
        nc.sync.dma_start(sinq_s[:], sinq[:, :])
        cosk_s = persist.tile([T, HD], f32, tag="cosk_s")
        nc.sync.dma_start(cosk_s[:], cosk[:, :])
        sink_s = persist.tile([T, HD], f32, tag="sink_s")
        nc.sync.dma_start(sink_s[:], sink[:, :])
        if use_mask:
            maskT_s = persist.tile([128, NLC * HS], f32, tag="maskT_s")
            nc.sync.dma_start(
                maskT_s[:].rearrange("p (c j) -> p c j", c=NLC),
                maskT[:, :].rearrange("(c p) j -> p c j", p=128),
            )

        # outputs of phase 1, used throughout
        xqT_s = persist.tile([128, QW], adt, tag="xqT_s")     # [hd, (h,t)]
        xk_rot = persist.tile([T, HD], f32, tag="xk_rot")     # [t, hd]
        xv_s = persist.tile([T, HD], f32, tag="xv_s")         # [t, hd]
        attnT = persist.tile([128, QW], pdt, tag="attnT")     # [hd, (h,t)]

        # ================= phase 1: projections + rotary =================
        with ExitStack() as ph1:
            wqp = ph1.enter_context(tc.tile_pool(name="wqp", bufs=3))
            wkvp = ph1.enter_context(tc.tile_pool(name="wkvp", bufs=1))
            rotp = ph1.enter_context(tc.tile_pool(name="rotp", bufs=1))
            pqkv = ph1.enter_context(tc.tile_pool(name="pqkv", bufs=1, space="PSUM"))

            ps_xq = pqkv.tile([T, QW], f32, tag="ps_xq")
            ps_xk = pqkv.tile([T, HD], f32, tag="ps_xk")
            ps_xv = pqkv.tile([T, HD], f32, tag="ps_xv")

            wk_t = wkvp.tile([128, NDC * HD], pdt, tag="wk_t")
            nc.sync.dma_start(
                wk_t[:].rearrange("p (c d) -> p c d", c=NDC),
                wkT[:, :].rearrange("(c p) d -> p c d", p=128),
            )
            wv_t = wkvp.tile([128, NDC * HD], pdt, tag="wv_t")
            nc.sync.dma_start(
                wv_t[:].rearrange("p (c d) -> p c d", c=NDC),
                wvT[:, :].rearrange("(c p) d -> p c d", p=128),
            )

            CPW = 4  # chunks per wq tile
            for j in range(NDC // CPW):
                wq_t = wqp.tile([128, CPW * QW], pdt, tag="wq_t")
                nc.sync.dma_start(
                    wq_t[:].rearrange("p (c d) -> p c d", c=CPW),
                    wqT[j * CPW * 128:(j + 1) * CPW * 128, :].rearrange(
                        "(c p) d -> p c d", p=128
                    ),
                )
                for jj in range(CPW):
                    c = j * CPW + jj
                    lhs = xT_s[:, c * T:(c + 1) * T]
                    nc.tensor.matmul(
                        ps_xq[:, :],
                        lhs,
                        wq_t[:, jj * QW:(jj + 1) * QW],
                        start=(c == 0),
                        stop=(c == NDC - 1),
                    )
            for c in range(NDC):
                lhs = xT_s[:, c * T:(c + 1) * T]
                nc.tensor.matmul(
                    ps_xk[:, :], lhs, wk_t[:, c * HD:(c + 1) * HD],
                    start=(c == 0), stop=(c == NDC - 1),
                )
            for c in range(NDC):
                lhs = xT_s[:, c * T:(c + 1) * T]
                nc.tensor.matmul(
                    ps_xv[:, :], lhs, wv_t[:, c * HD:(c + 1) * HD],
                    start=(c == 0), stop=(c == NDC - 1),
                )

            # rotary on q (scale folded into tables), in natural layout
            q_cos = rotp.tile([T, QW], f32, tag="q_cos")
            nc.vector.tensor_mul(q_cos[:], ps_xq[:], cosq_s[:])
            q_swp = rotp.tile([T, QW], f32, tag="q_swp")
            nc.vector.tensor_copy(q_swp[:, 0:QW:2], ps_xq[:, 1:QW:2])
            nc.vector.tensor_copy(q_swp[:, 1:QW:2], ps_xq[:, 0:QW:2])
            q_sin = rotp.tile([T, QW], f32, tag="q_sin")
            nc.vector.tensor_mul(q_sin[:], q_swp[:], sinq_s[:])
            xq_rot = rotp.tile([T, QW], f32, tag="xq_rot")
            nc.vector.tensor_add(xq_rot[:], q_cos[:], q_sin[:])

            # rotary on k
            k_cos = rotp.tile([T, HD], f32, tag="k_cos")
            nc.vector.tensor_mul(k_cos[:], ps_xk[:], cosk_s[:])
            k_swp = rotp.tile([T, HD], f32, tag="k_swp")
            nc.vector.tensor_copy(k_swp[:, 0:HD:2], ps_xk[:, 1:HD:2])
            nc.vector.tensor_copy(k_swp[:, 1:HD:2], ps_xk[:, 0:HD:2])
            k_sin = rotp.tile([T, HD], f32, tag="k_sin")
            nc.vector.tensor_mul(k_sin[:], k_swp[:], sink_s[:])
            nc.vector.tensor_add(xk_rot[:], k_cos[:], k_sin[:])

            nc.scalar.copy(xv_s[:], ps_xv[:])

            # transpose q per head -> xqT [hd, (h,t)]
            for h in range(HQ):
                ps_t = ptr.tile([128, 128], f32, tag="ps_t")
                nc.tensor.transpose(
                    ps_t[:, :], xq_rot[:, h * HD:(h + 1) * HD], ident[:, :]
                )
                nc.scalar.copy(xqT_s[:, h * T:(h + 1) * T], ps_t[:])

        # ================= phase 2: attention over batches =================
        with ExitStack() as ph2:
            kvp = ph2.enter_context(tc.tile_pool(name="kvp", bufs=3))
            kbfp = ph2.enter_context(tc.tile_pool(name="kbfp", bufs=2))
            ktp = ph2.enter_context(tc.tile_pool(name="ktp", bufs=6))
            prp = ph2.enter_context(tc.tile_pool(name="prp", bufs=2))
            bcp = ph2.enter_context(tc.tile_pool(name="bcp", bufs=2))
            psc = ph2.enter_context(tc.tile_pool(name="psc", bufs=2, space="PSUM"))
            ppv = ph2.enter_context(tc.tile_pool(name="ppv", bufs=2, space="PSUM"))
            psm = ph2.enter_context(tc.tile_pool(name="psm", bufs=1, space="PSUM"))
            pbc = ph2.enter_context(tc.tile_pool(name="pbc", bufs=1, space="PSUM"))
            ones_col = persist.tile([128, 1], adt, tag="ones_col")
            nc.gpsimd.memset(ones_col[:], 1.0)

            NF = NLC - 1  # 31 full chunks from the old cache
            pending = {}

            def _write_back(wb, wk_main, wk31, wv_main, wv31):
                nc.gpsimd.dma_start(
                    cko[wb, 0:NF * 128, :].rearrange("(c p) d -> p c d", p=128),
                    wk_main[:].rearrange("p (c d) -> p c d", c=NF),
                )
                nc.gpsimd.dma_start(cko[wb, NF * 128:L, :], wk31[:, :])
                nc.gpsimd.dma_start(
                    cvo[wb, 0:NF * 128, :].rearrange("(c p) d -> p c d", p=128),
                    wv_main[:].rearrange("p (c d) -> p c d", c=NF),
                )
                nc.gpsimd.dma_start(cvo[wb, NF * 128:L, :], wv31[:, :])

            for b in range(B):
                # --- load shifted cache window (k on sync ring, v on scalar) ---
                # loads via SWDGE: 16-lane descriptor generation on its own
                # queue rows; the HWDGE rings only carry writes/weights
                k_main = kvp.tile([128, NF * HD], f32, tag="k_main")
                nc.gpsimd.dma_start(
                    k_main[:].rearrange("p (c d) -> p c d", c=NF),
                    ck[b, S:S + NF * 128, :].rearrange("(c p) d -> p c d", p=128),
                )
                v_main = kvp.tile([128, NF * HD], f32, tag="v_main")
                nc.gpsimd.dma_start(
                    v_main[:].rearrange("p (c d) -> p c d", c=NF),
                    cv[b, S:S + NF * 128, :].rearrange("(c p) d -> p c d", p=128),
                )
                # last chunk: 112 old rows + 16 new rows
                k31 = kvp.tile([128, HD], f32, tag="k31")
                nc.gpsimd.dma_start(k31[0:112, :], ck[b, S + NF * 128:L, :])
                nc.scalar.dma_start(k31[112:128, :], xk_rot[b * S:(b + 1) * S, :])
                v31 = kvp.tile([128, HD], f32, tag="v31")
                nc.gpsimd.dma_start(v31[0:112, :], cv[b, S + NF * 128:L, :])
                nc.scalar.dma_start(v31[112:128, :], xv_s[b * S:(b + 1) * S, :])

                # write-back of the PREVIOUS batch via SWDGE: by now its
                # loads have long completed, so the gpsimd queue never stalls
                if b > 0:
                    _write_back(b - 1, *pending[b - 1])
                pending[b] = (k_main, k31, v_main, v31)

                # bf16 copies of k (for transposes) and v (for PV matmuls)
                if BF16_ATTN:
                    k_bf = kbfp.tile([128, NLC * HD], adt, tag="k_bf")
                    nc.vector.tensor_copy(k_bf[:, 0:NF * HD], k_main[:])
                    nc.vector.tensor_copy(k_bf[:, NF * HD:NLC * HD], k31[:])
                    v_bf = kbfp.tile([128, NLC * HD], adt, tag="v_bf")
                    nc.vector.tensor_copy(v_bf[:, 0:NF * HD], v_main[:])
                    nc.vector.tensor_copy(v_bf[:, NF * HD:NLC * HD], v31[:])

                # --- attention ---
                rhs_q = xqT_s[:].rearrange("p (h t) -> p h t", h=HQ)[
                    :, :, b * S:(b + 1) * S
                ]
                ps_pv = ppv.tile([128, HS], f32, tag="ps_pv")
                ps_sum = psm.tile([1, HS], f32, tag="ps_sum")

                for g in range(NLC // GRP):
                    ps_sc = psc.tile([128, GRP * HS], f32, tag="ps_sc")
                    for q4 in range(GRP // 4):
                        ps_t = ptr.tile([128, 4 * HD], adt, tag="ps_t")
                        for jj4 in range(4):
                            jj = q4 * 4 + jj4
                            c = g * GRP + jj
                            if BF16_ATTN:
                                ktile = k_bf[:, c * HD:(c + 1) * HD]
                            else:
                                ktile = k31[:, :] if c == NF else k_main[:, c * HD:(c + 1) * HD]
                            nc.tensor.transpose(
                                ps_t[:, jj4 * HD:(jj4 + 1) * HD], ktile, ident_a[:, :]
                            )
                        kT_t = ktp.tile([128, 4 * HD], adt, tag="kT_t")
                        nc.vector.tensor_copy(kT_t[:], ps_t[:])
                        for jj4 in range(4):
                            jj = q4 * 4 + jj4
                            nc.tensor.matmul(
                                ps_sc[:, jj * HS:(jj + 1) * HS],
                                kT_t[:, jj4 * HD:(jj4 + 1) * HD],
                                rhs_q,
                                start=True,
                                stop=True,
                            )
                    probs = prp.tile([128, GRP * HS], adt, tag="probs")
                    if use_mask:
                        pm = prp.tile([128, GRP * HS], f32, tag="pm")
                        nc.vector.tensor_add(
                            pm[:], ps_sc[:],
                            maskT_s[:, g * GRP * HS:(g + 1) * GRP * HS],
                        )
                        nc.scalar.activation(
                            probs[:], pm[:], mybir.ActivationFunctionType.Exp,
                        )
                    else:
                        nc.scalar.activation(
                            probs[:], ps_sc[:], mybir.ActivationFunctionType.Exp,
                        )
                    for jj in range(GRP):
                        c = g * GRP + jj
                        if BF16_ATTN:
                            vtile = v_bf[:, c * HD:(c + 1) * HD]
                        else:
                            vtile = v31[:, :] if c == NF else v_main[:, c * HD:(c + 1) * HD]
                        pslice = probs[:, jj * HS:(jj + 1) * HS]
                        nc.tensor.matmul(
                            ps_pv[:, :], vtile, pslice,
                            start=(c == 0), stop=(c == NLC - 1),
                        )
                        nc.tensor.matmul(
                            ps_sum[:, :], ones_col[:], pslice,
                            start=(c == 0), stop=(c == NLC - 1),
                        )

                rec = bcp.tile([1, HS], f32, tag="rec")
                nc.vector.reciprocal(rec[:], ps_sum[:])
                ps_bc = pbc.tile([128, HS], f32, tag="ps_bc")
                nc.tensor.matmul(ps_bc[:, :], ones_row[:], rec[:], start=True, stop=True)
                bc_s = bcp.tile([128, HS], f32, tag="bc_s")
                nc.scalar.copy(bc_s[:], ps_bc[:])
                attn_dst = attnT[:].rearrange("p (h t) -> p h t", h=HQ)[
                    :, :, b * S:(b + 1) * S
                ]
                nc.vector.tensor_mul(attn_dst, ps_pv[:], bc_s[:])

            _write_back(B - 1, *pending[B - 1])

        # ================= phase 3: output projection + RS =================
        with ExitStack() as ph3:
            wop = ph3.enter_context(tc.tile_pool(name="wop", bufs=4))
            outp = ph3.enter_context(tc.tile_pool(name="outp", bufs=1))
            pout = ph3.enter_context(tc.tile_pool(name="pout", bufs=2, space="PSUM"))

            wo_tiles = []
            for h in range(HQ):
                wo_t = wop.tile([128, D], pdt, tag="wo_t")
                eng = nc.sync if h % 2 == 0 else nc.scalar
                eng.dma_start(wo_t[:], woT[h * HD:(h + 1) * HD, :])
                wo_tiles.append(wo_t)

            out_s = outp.tile([T, D], f32, tag="out_s")
            NO = D // 512
            NRS = 2
            for half in range(NRS):
                for nn in range(NO // NRS):
                    n = half * (NO // NRS) + nn
                    ps_o = pout.tile([T, 512], f32, tag="ps_o")
                    for h in range(HQ):
                        nc.tensor.matmul(
                            ps_o[:, :],
                            attnT[:, h * T:(h + 1) * T],
                            wo_tiles[h][:, n * 512:(n + 1) * 512],
                            start=(h == 0),
                            stop=(h == HQ - 1),
                        )
                    if n % 2 == 0:
                        nc.vector.tensor_copy(out_s[:, n * 512:(n + 1) * 512], ps_o[:])
                    else:
                        nc.scalar.copy(out_s[:, n * 512:(n + 1) * 512], ps_o[:])
                DH = D // NRS
                nc.sync.dma_start(
                    rs_in[half][:, :], out_s[:, half * DH:(half + 1) * DH]
                )
                nc.gpsimd.collective_compute(
                    "ReduceScatter",
                    mybir.AluOpType.add,
                    ins=[rs_in[half][:, :]],
                    outs=[rs_out[half][:, :]],
                    replica_groups=[list(range(NCORES))],
                )
                nc.sync.dma_start(y[:, half * DH:(half + 1) * DH], rs_out[half][:, :])

    nc.compile()
    return nc


def _get_nc(use_mask):
    key = ("nc", use_mask)
    if key not in _CACHE:
        _CACHE[key] = _build(use_mask)
    return _CACHE[key]


def _prep_inputs(x, mask, freqs_cis, cache_k, cache_v, wq, wk, wv, wo):
    import ml_dtypes

    pnp = ml_dtypes.bfloat16 if BF16_PROJ else np.float32

    def f(a):
        return np.ascontiguousarray(a)

    def fp(a):
        return np.ascontiguousarray(a, dtype=pnp)

    x = np.asarray(x, np.float32)
    mask = np.asarray(mask, np.float32)
    fc = np.asarray(freqs_cis, np.float32)
    cache_k = np.asarray(cache_k, np.float32)
    cache_v = np.asarray(cache_v, np.float32)
    wq = np.asarray(wq, np.float32)
    wk = np.asarray(wk, np.float32)
    wv = np.asarray(wv, np.float32)
    wo = np.asarray(wo, np.float32)

    xT = fp(x.reshape(T, D).T)

    cos = fc[:, :, 0]
    sin = fc[:, :, 1]
    cos2 = np.repeat(cos, 2, axis=1)            # [S, HD]
    sin2 = np.empty((S, HD), np.float32)
    sin2[:, 0::2] = -sin
    sin2[:, 1::2] = sin
    isq = np.float32(1.0 / math.sqrt(HD))
    cosq = np.tile(np.tile(cos2, (B, 1)), (1, HQ)) * isq   # [T, QW]
    sinq = np.tile(np.tile(sin2, (B, 1)), (1, HQ)) * isq
    cosk = f(np.tile(cos2, (B, 1)))                        # [T, HD]
    sink = f(np.tile(sin2, (B, 1)))
    use_mask = bool(np.any(mask))
    maskT = f(np.tile(mask.T, (1, HQ))) if use_mask else None  # [L, HQ*S]

    wqT_full = wq.T          # [D, NH*HD]
    wkT_full = wk.T
    wvT_full = wv.T
    woT_full = wo.T          # [NH*HD, D]

    in_maps = []
    for i in range(NCORES):
        in_maps.append({
            "xT": xT,
            "wqT": fp(wqT_full[:, i * QW:(i + 1) * QW]),
            "wkT": fp(wkT_full[:, i * HD:(i + 1) * HD]),
            "wvT": fp(wvT_full[:, i * HD:(i + 1) * HD]),
            "woT": fp(woT_full[i * QW:(i + 1) * QW, :]),
            "ck": f(cache_k[:, :, i, :]),
            "cv": f(cache_v[:, :, i, :]),
            "cosq": f(cosq),
            "sinq": f(sinq),
            "cosk": cosk,
            "sink": sink,
        })
        if use_mask:
            in_maps[-1]["maskT"] = maskT
    return in_maps, use_mask


def kernel(x, mask, freqs_cis, cache_k, cache_v, wq, wk, wv, wo):
    from concourse.bass_utils import run_bass_kernel_spmd

    in_maps, use_mask = _prep_inputs(
        x, mask, freqs_cis, cache_k, cache_v, wq, wk, wv, wo
    )
    nc = _get_nc(use_mask)
    res = run_bass_kernel_spmd(nc, in_maps, core_ids=list(range(NCORES)))
    outs = res.results

    out = np.empty((B, S, D), np.float32)
    for i in range(NCORES):
        out[i] = outs[i]["y"]
    ck_new = np.stack([outs[i]["cko"] for i in range(NCORES)], axis=2)
    cv_new = np.stack([outs[i]["cvo"] for i in range(NCORES)], axis=2)
    return out, ck_new, cv_new


# revision 21
# speedup vs baseline: 1.2098x; 1.2098x over previous
"""Trainium2 Bass kernel for nn_Attention_1486058684451.

Decode-style attention with sliding-window KV-cache update, sharded
tensor-parallel over 8 NeuronCores: each core owns 4 query heads + 1 KV
head (wq/wk/wv output rows, wo input columns, cache KV-head slice).
ReduceScatter combines the wo partials; the host concatenates shards.

All shapes hardcoded per the problem spec:
  x[8,16,4096], mask[16,4096], freqs_cis[16,64,2],
  cache_k/v[8,4096,8,128], wq[4096,4096], wk/wv[1024,4096], wo[4096,4096]
"""

import math
from contextlib import ExitStack

import numpy as np

B, S, D = 8, 16, 4096
NH, NKV, HD = 32, 8, 128
L = 4096
NCORES = 8
T = B * S                   # 128 tokens
HQ = NH // NCORES           # 4 local q heads
QW = HQ * HD                # 512 local q-projection width
NDC = D // 128              # 32 contraction chunks
NLC = L // 128              # 32 cache chunks
GRP = 8                     # score chunks packed per PSUM bank
HS = HQ * S                 # 64 score columns per (head, seq)

BF16_ATTN = True            # scores + PV matmuls in bf16 (fp32 accumulate)
BF16_PROJ = True            # q/k/v/o projections in bf16 (weights host-cast)

_CACHE = {}


def _build(use_mask):
    import concourse.tile as tile
    from concourse import bacc, bass_isa, mybir
    from concourse.masks import make_identity

    f32 = mybir.dt.float32
    bf16 = mybir.dt.bfloat16
    adt = bf16 if BF16_ATTN else f32     # attention operand dtype
    pdt = bf16 if BF16_PROJ else f32     # projection operand dtype

    nc = bacc.Bacc(
        "TRN2",
        target_bir_lowering=False,
        debug=False,
        num_devices=NCORES,
    )

    # ---- kernel I/O (per-core shards, host-prepared) ----
    xT = nc.dram_tensor("xT", [D, T], pdt, kind="ExternalInput")
    wqT = nc.dram_tensor("wqT", [D, QW], pdt, kind="ExternalInput")
    wkT = nc.dram_tensor("wkT", [D, HD], pdt, kind="ExternalInput")
    wvT = nc.dram_tensor("wvT", [D, HD], pdt, kind="ExternalInput")
    woT = nc.dram_tensor("woT", [QW, D], pdt, kind="ExternalInput")
    ck = nc.dram_tensor("ck", [B, L, HD], f32, kind="ExternalInput")
    cv = nc.dram_tensor("cv", [B, L, HD], f32, kind="ExternalInput")
    cosq = nc.dram_tensor("cosq", [T, QW], f32, kind="ExternalInput")
    sinq = nc.dram_tensor("sinq", [T, QW], f32, kind="ExternalInput")
    cosk = nc.dram_tensor("cosk", [T, HD], f32, kind="ExternalInput")
    sink = nc.dram_tensor("sink", [T, HD], f32, kind="ExternalInput")
    maskT = (
        nc.dram_tensor("maskT", [L, HS], f32, kind="ExternalInput")
        if use_mask else None
    )

    y = nc.dram_tensor("y", [S, D], f32, kind="ExternalOutput")
    cko = nc.dram_tensor("cko", [B, L, HD], f32, kind="ExternalOutput")
    cvo = nc.dram_tensor("cvo", [B, L, HD], f32, kind="ExternalOutput")

    NRS = 2
    DH = D // NRS
    rs_in = [nc.dram_tensor(f"rs_in{j}", [T, DH], f32) for j in range(NRS)]
    rs_out = [nc.dram_tensor(f"rs_out{j}", [S, DH], f32) for j in range(NRS)]

    with tile.TileContext(nc) as tc, ExitStack() as top:
        persist = top.enter_context(tc.tile_pool(name="persist", bufs=1))
        ptr = top.enter_context(tc.tile_pool(name="ptr", bufs=2, space="PSUM"))

        ident = persist.tile([128, 128], f32, tag="ident")
        make_identity(nc, ident[:])
        ident_a = persist.tile([128, 128], adt, tag="ident_a")
        nc.vector.tensor_copy(ident_a[:], ident[:])
        ones_row = persist.tile([1, 128], f32, tag="ones_row")
        nc.gpsimd.memset(ones_row[:], 1.0)

        xT_s = persist.tile([128, NDC * T], pdt, tag="xT_s")
        nc.sync.dma_start(
            xT_s[:].rearrange("p (c t) -> p c t", c=NDC),
            xT[:, :].rearrange("(c p) t -> p c t", p=128),
        )
        cosq_s = persist.tile([T, QW], f32, tag="cosq_s")
        nc.sync.dma_start(cosq_s[:], cosq[:, :])
        sinq_s = persist.tile([T, QW], f32, tag="sinq_s")
        nc.sync.dma_start(sinq_s[:], sinq[:, :])
        cosk_s = persist.tile([T, HD], f32, tag="cosk_s")
        nc.sync.dma_start(cosk_s[:], cosk[:, :])
        sink_s = persist.tile([T, HD], f32, tag="sink_s")
        nc.sync.dma_start(sink_s[:], sink[:, :])
        if use_mask:
            maskT_s = persist.tile([128, NLC * HS], f32, tag="maskT_s")
            nc.sync.dma_start(
                maskT_s[:].rearrange("p (c j) -> p c j", c=NLC),
                maskT[:, :].rearrange("(c p) j -> p c j", p=128),
            )

        # outputs of phase 1, used throughout
        xqT_s = persist.tile([128, QW], adt, tag="xqT_s")     # [hd, (h,t)]
        xk_rot = persist.tile([T, HD], f32, tag="xk_rot")     # [t, hd]
        xv_s = persist.tile([T, HD], f32, tag="xv_s")         # [t, hd]
        attnT = persist.tile([128, QW], pdt, tag="attnT")     # [hd, (h,t)]

        # ================= phase 1: projections + rotary =================
        with ExitStack() as ph1:
            wqp = ph1.enter_context(tc.tile_pool(name="wqp", bufs=3))
            wkvp = ph1.enter_context(tc.tile_pool(name="wkvp", bufs=1))
            rotp = ph1.enter_context(tc.tile_pool(name="rotp", bufs=1))
            pqkv = ph1.enter_context(tc.tile_pool(name="pqkv", bufs=1, space="PSUM"))

            ps_xq = pqkv.tile([T, QW], f32, tag="ps_xq")
            ps_xk = pqkv.tile([T, HD], f32, tag="ps_xk")
            ps_xv = pqkv.tile([T, HD], f32, tag="ps_xv")

            wk_t = wkvp.tile([128, NDC * HD], pdt, tag="wk_t")
            nc.sync.dma_start(
                wk_t[:].rearrange("p (c d) -> p c d", c=NDC),
                wkT[:, :].rearrange("(c p) d -> p c d", p=128),
            )
            wv_t = wkvp.tile([128, NDC * HD], pdt, tag="wv_t")
            nc.sync.dma_start(
                wv_t[:].rearrange("p (c d) -> p c d", c=NDC),
                wvT[:, :].rearrange("(c p) d -> p c d", p=128),
            )

            CPW = 4  # chunks per wq tile
            for j in range(NDC // CPW):
                wq_t = wqp.tile([128, CPW * QW], pdt, tag="wq_t")
                nc.sync.dma_start(
                    wq_t[:].rearrange("p (c d) -> p c d", c=CPW),
                    wqT[j * CPW * 128:(j + 1) * CPW * 128, :].rearrange(
                        "(c p) d -> p c d", p=128
                    ),
                )
                for jj in range(CPW):
                    c = j * CPW + jj
                    lhs = xT_s[:, c * T:(c + 1) * T]
                    nc.tensor.matmul(
                        ps_xq[:, :],
                        lhs,
                        wq_t[:, jj * QW:(jj + 1) * QW],
                        start=(c == 0),
                        stop=(c == NDC - 1),
                    )
            for c in range(NDC):
                lhs = xT_s[:, c * T:(c + 1) * T]
                nc.tensor.matmul(
                    ps_xk[:, :], lhs, wk_t[:, c * HD:(c + 1) * HD],
                    start=(c == 0), stop=(c == NDC - 1),
                )
            for c in range(NDC):
                lhs = xT_s[:, c * T:(c + 1) * T]
                nc.tensor.matmul(
                    ps_xv[:, :], lhs, wv_t[:, c * HD:(c + 1) * HD],
                    start=(c == 0), stop=(c == NDC - 1),
                )

            # rotary on q (scale folded into tables), in natural layout
            q_cos = rotp.tile([T, QW], f32, tag="q_cos")
            nc.vector.tensor_mul(q_cos[:], ps_xq[:], cosq_s[:])
            q_swp = rotp.tile([T, QW], f32, tag="q_swp")
            nc.vector.tensor_copy(q_swp[:, 0:QW:2], ps_xq[:, 1:QW:2])
            nc.vector.tensor_copy(q_swp[:, 1:QW:2], ps_xq[:, 0:QW:2])
            q_sin = rotp.tile([T, QW], f32, tag="q_sin")
            nc.vector.tensor_mul(q_sin[:], q_swp[:], sinq_s[:])
            xq_rot = rotp.tile([T, QW], f32, tag="xq_rot")
            nc.vector.tensor_add(xq_rot[:], q_cos[:], q_sin[:])

            # rotary on k
            k_cos = rotp.tile([T, HD], f32, tag="k_cos")
            nc.vector.tensor_mul(k_cos[:], ps_xk[:], cosk_s[:])
            k_swp = rotp.tile([T, HD], f32, tag="k_swp")
            nc.vector.tensor_copy(k_swp[:, 0:HD:2], ps_xk[:, 1:HD:2])
            nc.vector.tensor_copy(k_swp[:, 1:HD:2], ps_xk[:, 0:HD:2])
            k_sin = rotp.tile([T, HD], f32, tag="k_sin")
            nc.vector.tensor_mul(k_sin[:], k_swp[:], sink_s[:])
            nc.vector.tensor_add(xk_rot[:], k_cos[:], k_sin[:])

            nc.scalar.copy(xv_s[:], ps_xv[:])

            # transpose q per head -> xqT [hd, (h,t)]
            for h in range(HQ):
                ps_t = ptr.tile([128, 128], f32, tag="ps_t")
                nc.tensor.transpose(
                    ps_t[:, :], xq_rot[:, h * HD:(h + 1) * HD], ident[:, :]
                )
                nc.scalar.copy(xqT_s[:, h * T:(h + 1) * T], ps_t[:])

        # ================= phase 2: attention over batches =================
        with ExitStack() as ph2:
            kvp = ph2.enter_context(tc.tile_pool(name="kvp", bufs=3))
            kbfp = ph2.enter_context(tc.tile_pool(name="kbfp", bufs=2))
            ktp = ph2.enter_context(tc.tile_pool(name="ktp", bufs=6))
            prp = ph2.enter_context(tc.tile_pool(name="prp", bufs=2))
            bcp = ph2.enter_context(tc.tile_pool(name="bcp", bufs=2))
            psc = ph2.enter_context(tc.tile_pool(name="psc", bufs=2, space="PSUM"))
            ppv = ph2.enter_context(tc.tile_pool(name="ppv", bufs=2, space="PSUM"))
            psm = ph2.enter_context(tc.tile_pool(name="psm", bufs=1, space="PSUM"))
            pbc = ph2.enter_context(tc.tile_pool(name="pbc", bufs=1, space="PSUM"))
            ones_col = persist.tile([128, 1], adt, tag="ones_col")
            nc.gpsimd.memset(ones_col[:], 1.0)

            NF = NLC - 1  # 31 full chunks from the old cache
            for b in range(B):
                # --- load shifted cache window (k on sync ring, v on scalar) ---
                # loads via SWDGE: 16-lane descriptor generation on its own
                # queue rows; the HWDGE rings only carry writes/weights
                k_main = kvp.tile([128, NF * HD], f32, tag="k_main")
                nc.gpsimd.dma_start(
                    k_main[:].rearrange("p (c d) -> p c d", c=NF),
                    ck[b, S:S + NF * 128, :].rearrange("(c p) d -> p c d", p=128),
                )
                v_main = kvp.tile([128, NF * HD], f32, tag="v_main")
                nc.gpsimd.dma_start(
                    v_main[:].rearrange("p (c d) -> p c d", c=NF),
                    cv[b, S:S + NF * 128, :].rearrange("(c p) d -> p c d", p=128),
                )
                # last chunk: 112 old rows + 16 new rows
                k31 = kvp.tile([128, HD], f32, tag="k31")
                nc.gpsimd.dma_start(k31[0:112, :], ck[b, S + NF * 128:L, :])
                nc.scalar.dma_start(k31[112:128, :], xk_rot[b * S:(b + 1) * S, :])
                v31 = kvp.tile([128, HD], f32, tag="v31")
                nc.gpsimd.dma_start(v31[0:112, :], cv[b, S + NF * 128:L, :])
                nc.scalar.dma_start(v31[112:128, :], xv_s[b * S:(b + 1) * S, :])

                # --- write updated cache back (HWDGE, k on SP / v on ACT) ---
                nc.sync.dma_start(
                    cko[b, 0:NF * 128, :].rearrange("(c p) d -> p c d", p=128),
                    k_main[:].rearrange("p (c d) -> p c d", c=NF),
                )
                nc.sync.dma_start(cko[b, NF * 128:L, :], k31[:, :])
                nc.scalar.dma_start(
                    cvo[b, 0:NF * 128, :].rearrange("(c p) d -> p c d", p=128),
                    v_main[:].rearrange("p (c d) -> p c d", c=NF),
                )
                nc.scalar.dma_start(cvo[b, NF * 128:L, :], v31[:, :])

                # bf16 copies of k (for transposes) and v (for PV matmuls)
                if BF16_ATTN:
                    k_bf = kbfp.tile([128, NLC * HD], adt, tag="k_bf")
                    nc.vector.tensor_copy(k_bf[:, 0:NF * HD], k_main[:])
                    nc.vector.tensor_copy(k_bf[:, NF * HD:NLC * HD], k31[:])
                    v_bf = kbfp.tile([128, NLC * HD], adt, tag="v_bf")
                    nc.vector.tensor_copy(v_bf[:, 0:NF * HD], v_main[:])
                    nc.vector.tensor_copy(v_bf[:, NF * HD:NLC * HD], v31[:])

                # --- attention ---
                rhs_q = xqT_s[:].rearrange("p (h t) -> p h t", h=HQ)[
                    :, :, b * S:(b + 1) * S
                ]
                ps_pv = ppv.tile([128, HS], f32, tag="ps_pv")
                ps_sum = psm.tile([1, HS], f32, tag="ps_sum")

                for g in range(NLC // GRP):
                    ps_sc = psc.tile([128, GRP * HS], f32, tag="ps_sc")
                    for q4 in range(GRP // 4):
                        ps_t = ptr.tile([128, 4 * HD], adt, tag="ps_t")
                        for jj4 in range(4):
                            jj = q4 * 4 + jj4
                            c = g * GRP + jj
                            if BF16_ATTN:
                                ktile = k_bf[:, c * HD:(c + 1) * HD]
                            else:
                                ktile = k31[:, :] if c == NF else k_main[:, c * HD:(c + 1) * HD]
                            nc.tensor.transpose(
                                ps_t[:, jj4 * HD:(jj4 + 1) * HD], ktile, ident_a[:, :]
                            )
                        kT_t = ktp.tile([128, 4 * HD], adt, tag="kT_t")
                        nc.vector.tensor_copy(kT_t[:], ps_t[:])
                        for jj4 in range(4):
                            jj = q4 * 4 + jj4
                            nc.tensor.matmul(
                                ps_sc[:, jj * HS:(jj + 1) * HS],
                                kT_t[:, jj4 * HD:(jj4 + 1) * HD],
                                rhs_q,
                                start=True,
                                stop=True,
                            )
                    probs = prp.tile([128, GRP * HS], adt, tag="probs")
                    if use_mask:
                        pm = prp.tile([128, GRP * HS], f32, tag="pm")
                        nc.vector.tensor_add(
                            pm[:], ps_sc[:],
                            maskT_s[:, g * GRP * HS:(g + 1) * GRP * HS],
                        )
                        nc.scalar.activation(
                            probs[:], pm[:], mybir.ActivationFunctionType.Exp,
                        )
                    else:
                        nc.scalar.activation(
                            probs[:], ps_sc[:], mybir.ActivationFunctionType.Exp,
                        )
                    for jj in range(GRP):
                        c = g * GRP + jj
                        if BF16_ATTN:
                            vtile = v_bf[:, c * HD:(c + 1) * HD]
                        else:
                            vtile = v31[:, :] if c == NF else v_main[:, c * HD:(c + 1) * HD]
                        pslice = probs[:, jj * HS:(jj + 1) * HS]
                        nc.tensor.matmul(
                            ps_pv[:, :], vtile, pslice,
                            start=(c == 0), stop=(c == NLC - 1),
                        )
                        nc.tensor.matmul(
                            ps_sum[:, :], ones_col[:], pslice,
                            start=(c == 0), stop=(c == NLC - 1),
                        )

                rec = bcp.tile([1, HS], f32, tag="rec")
                nc.vector.reciprocal(rec[:], ps_sum[:])
                ps_bc = pbc.tile([128, HS], f32, tag="ps_bc")
                nc.tensor.matmul(ps_bc[:, :], ones_row[:], rec[:], start=True, stop=True)
                bc_s = bcp.tile([128, HS], f32, tag="bc_s")
                nc.scalar.copy(bc_s[:], ps_bc[:])
                attn_dst = attnT[:].rearrange("p (h t) -> p h t", h=HQ)[
                    :, :, b * S:(b + 1) * S
                ]
                nc.vector.tensor_mul(attn_dst, ps_pv[:], bc_s[:])

        # ================= phase 3: output projection + RS =================
        with ExitStack() as ph3:
            wop = ph3.enter_context(tc.tile_pool(name="wop", bufs=4))
            outp = ph3.enter_context(tc.tile_pool(name="outp", bufs=1))
            pout = ph3.enter_context(tc.tile_pool(name="pout", bufs=2, space="PSUM"))

            wo_tiles = []
            for h in range(HQ):
                wo_t = wop.tile([128, D], pdt, tag="wo_t")
                eng = nc.sync if h % 2 == 0 else nc.scalar
                eng.dma_start(wo_t[:], woT[h * HD:(h + 1) * HD, :])
                wo_tiles.append(wo_t)

            out_s = outp.tile([T, D], f32, tag="out_s")
            NO = D // 512
            NRS = 2
            for half in range(NRS):
                for nn in range(NO // NRS):
                    n = half * (NO // NRS) + nn
                    ps_o = pout.tile([T, 512], f32, tag="ps_o")
                    for h in range(HQ):
                        nc.tensor.matmul(
                            ps_o[:, :],
                            attnT[:, h * T:(h + 1) * T],
                            wo_tiles[h][:, n * 512:(n + 1) * 512],
                            start=(h == 0),
                            stop=(h == HQ - 1),
                        )
                    if n % 2 == 0:
                        nc.vector.tensor_copy(out_s[:, n * 512:(n + 1) * 512], ps_o[:])
                    else:
                        nc.scalar.copy(out_s[:, n * 512:(n + 1) * 512], ps_o[:])
                DH = D // NRS
                nc.sync.dma_start(
                    rs_in[half][:, :], out_s[:, half * DH:(half + 1) * DH]
                )
                nc.gpsimd.collective_compute(
                    "ReduceScatter",
                    mybir.AluOpType.add,
                    ins=[rs_in[half][:, :]],
                    outs=[rs_out[half][:, :]],
                    replica_groups=[list(range(NCORES))],
                )
                nc.sync.dma_start(y[:, half * DH:(half + 1) * DH], rs_out[half][:, :])

    nc.compile()
    return nc


def _get_nc(use_mask):
    key = ("nc", use_mask)
    if key not in _CACHE:
        _CACHE[key] = _build(use_mask)
    return _CACHE[key]


def _prep_inputs(x, mask, freqs_cis, cache_k, cache_v, wq, wk, wv, wo):
    import ml_dtypes

    pnp = ml_dtypes.bfloat16 if BF16_PROJ else np.float32

    def f(a):
        return np.ascontiguousarray(a)

    def fp(a):
        return np.ascontiguousarray(a, dtype=pnp)

    x = np.asarray(x, np.float32)
    mask = np.asarray(mask, np.float32)
    fc = np.asarray(freqs_cis, np.float32)
    cache_k = np.asarray(cache_k, np.float32)
    cache_v = np.asarray(cache_v, np.float32)
    wq = np.asarray(wq, np.float32)
    wk = np.asarray(wk, np.float32)
    wv = np.asarray(wv, np.float32)
    wo = np.asarray(wo, np.float32)

    xT = fp(x.reshape(T, D).T)

    cos = fc[:, :, 0]
    sin = fc[:, :, 1]
    cos2 = np.repeat(cos, 2, axis=1)            # [S, HD]
    sin2 = np.empty((S, HD), np.float32)
    sin2[:, 0::2] = -sin
    sin2[:, 1::2] = sin
    isq = np.float32(1.0 / math.sqrt(HD))
    cosq = np.tile(np.tile(cos2, (B, 1)), (1, HQ)) * isq   # [T, QW]
    sinq = np.tile(np.tile(sin2, (B, 1)), (1, HQ)) * isq
    cosk = f(np.tile(cos2, (B, 1)))                        # [T, HD]
    sink = f(np.tile(sin2, (B, 1)))
    use_mask = bool(np.any(mask))
    maskT = f(np.tile(mask.T, (1, HQ))) if use_mask else None  # [L, HQ*S]

    wqT_full = wq.T          # [D, NH*HD]
    wkT_full = wk.T
    wvT_full = wv.T
    woT_full = wo.T          # [NH*HD, D]

    in_maps = []
    for i in range(NCORES):
        in_maps.append({
            "xT": xT,
            "wqT": fp(wqT_full[:, i * QW:(i + 1) * QW]),
            "wkT": fp(wkT_full[:, i * HD:(i + 1) * HD]),
            "wvT": fp(wvT_full[:, i * HD:(i + 1) * HD]),
            "woT": fp(woT_full[i * QW:(i + 1) * QW, :]),
            "ck": f(cache_k[:, :, i, :]),
            "cv": f(cache_v[:, :, i, :]),
            "cosq": f(cosq),
            "sinq": f(sinq),
            "cosk": cosk,
            "sink": sink,
        })
        if use_mask:
            in_maps[-1]["maskT"] = maskT
    return in_maps, use_mask


def kernel(x, mask, freqs_cis, cache_k, cache_v, wq, wk, wv, wo):
    from concourse.bass_utils import run_bass_kernel_spmd

    in_maps, use_mask = _prep_inputs(
        x, mask, freqs_cis, cache_k, cache_v, wq, wk, wv, wo
    )
    nc = _get_nc(use_mask)
    res = run_bass_kernel_spmd(nc, in_maps, core_ids=list(range(NCORES)))
    outs = res.results

    out = np.empty((B, S, D), np.float32)
    for i in range(NCORES):
        out[i] = outs[i]["y"]
    ck_new = np.stack([outs[i]["cko"] for i in range(NCORES)], axis=2)
    cv_new = np.stack([outs[i]["cvo"] for i in range(NCORES)], axis=2)
    return out, ck_new, cv_new


# revision 22
# speedup vs baseline: 1.2390x; 1.0242x over previous
"""Trainium2 Bass kernel for nn_Attention_1486058684451.

Decode-style attention with sliding-window KV-cache update, sharded
tensor-parallel over 8 NeuronCores: each core owns 4 query heads + 1 KV
head (wq/wk/wv output rows, wo input columns, cache KV-head slice).
ReduceScatter combines the wo partials; the host concatenates shards.

All shapes hardcoded per the problem spec:
  x[8,16,4096], mask[16,4096], freqs_cis[16,64,2],
  cache_k/v[8,4096,8,128], wq[4096,4096], wk/wv[1024,4096], wo[4096,4096]
"""

import math
from contextlib import ExitStack

import numpy as np

B, S, D = 8, 16, 4096
NH, NKV, HD = 32, 8, 128
L = 4096
NCORES = 8
T = B * S                   # 128 tokens
HQ = NH // NCORES           # 4 local q heads
QW = HQ * HD                # 512 local q-projection width
NDC = D // 128              # 32 contraction chunks
NLC = L // 128              # 32 cache chunks
GRP = 8                     # score chunks packed per PSUM bank
HS = HQ * S                 # 64 score columns per (head, seq)

BF16_ATTN = True            # scores + PV matmuls in bf16 (fp32 accumulate)
BF16_PROJ = True            # q/k/v/o projections in bf16 (weights host-cast)

_CACHE = {}


def _build(use_mask):
    import concourse.tile as tile
    from concourse import bacc, bass_isa, mybir
    from concourse.masks import make_identity

    f32 = mybir.dt.float32
    bf16 = mybir.dt.bfloat16
    adt = bf16 if BF16_ATTN else f32     # attention operand dtype
    pdt = bf16 if BF16_PROJ else f32     # projection operand dtype

    nc = bacc.Bacc(
        "TRN2",
        target_bir_lowering=False,
        debug=False,
        num_devices=NCORES,
    )

    # ---- kernel I/O (per-core shards, host-prepared) ----
    xT = nc.dram_tensor("xT", [D, T], pdt, kind="ExternalInput")
    wqT = nc.dram_tensor("wqT", [D, QW], pdt, kind="ExternalInput")
    wkT = nc.dram_tensor("wkT", [D, HD], pdt, kind="ExternalInput")
    wvT = nc.dram_tensor("wvT", [D, HD], pdt, kind="ExternalInput")
    woT = nc.dram_tensor("woT", [QW, D], pdt, kind="ExternalInput")
    ck = nc.dram_tensor("ck", [B, L, HD], f32, kind="ExternalInput")
    cv = nc.dram_tensor("cv", [B, L, HD], f32, kind="ExternalInput")
    cosq = nc.dram_tensor("cosq", [T, QW], f32, kind="ExternalInput")
    sinq = nc.dram_tensor("sinq", [T, QW], f32, kind="ExternalInput")
    cosk = nc.dram_tensor("cosk", [T, HD], f32, kind="ExternalInput")
    sink = nc.dram_tensor("sink", [T, HD], f32, kind="ExternalInput")
    maskT = (
        nc.dram_tensor("maskT", [L, HS], f32, kind="ExternalInput")
        if use_mask else None
    )

    y = nc.dram_tensor("y", [S, D], f32, kind="ExternalOutput")
    cko = nc.dram_tensor("cko", [B, L, HD], f32, kind="ExternalOutput")
    cvo = nc.dram_tensor("cvo", [B, L, HD], f32, kind="ExternalOutput")

    NRS = 2
    DH = D // NRS
    rs_in = [nc.dram_tensor(f"rs_in{j}", [T, DH], f32) for j in range(NRS)]
    rs_out = [nc.dram_tensor(f"rs_out{j}", [S, DH], f32) for j in range(NRS)]

    with tile.TileContext(nc) as tc, ExitStack() as top:
        persist = top.enter_context(tc.tile_pool(name="persist", bufs=1))
        ptr = top.enter_context(tc.tile_pool(name="ptr", bufs=2, space="PSUM"))

        ident = persist.tile([128, 128], f32, tag="ident")
        make_identity(nc, ident[:])
        ident_a = persist.tile([128, 128], adt, tag="ident_a")
        nc.vector.tensor_copy(ident_a[:], ident[:])
        ones_row = persist.tile([1, 128], f32, tag="ones_row")
        nc.gpsimd.memset(ones_row[:], 1.0)

        xT_s = persist.tile([128, NDC * T], pdt, tag="xT_s")
        nc.sync.dma_start(
            xT_s[:].rearrange("p (c t) -> p c t", c=NDC),
            xT[:, :].rearrange("(c p) t -> p c t", p=128),
        )
        cosq_s = persist.tile([T, QW], f32, tag="cosq_s")
        nc.sync.dma_start(cosq_s[:], cosq[:, :])
        sinq_s = persist.tile([T, QW], f32, tag="sinq_s")
        nc.sync.dma_start(sinq_s[:], sinq[:, :])
        cosk_s = persist.tile([T, HD], f32, tag="cosk_s")
        nc.sync.dma_start(cosk_s[:], cosk[:, :])
        sink_s = persist.tile([T, HD], f32, tag="sink_s")
        nc.sync.dma_start(sink_s[:], sink[:, :])
        if use_mask:
            maskT_s = persist.tile([128, NLC * HS], f32, tag="maskT_s")
            nc.sync.dma_start(
                maskT_s[:].rearrange("p (c j) -> p c j", c=NLC),
                maskT[:, :].rearrange("(c p) j -> p c j", p=128),
            )

        # outputs of phase 1, used throughout
        xqT_s = persist.tile([128, QW], adt, tag="xqT_s")     # [hd, (h,t)]
        xk_rot = persist.tile([T, HD], f32, tag="xk_rot")     # [t, hd]
        xv_s = persist.tile([T, HD], f32, tag="xv_s")         # [t, hd]
        attnT = persist.tile([128, QW], pdt, tag="attnT")     # [hd, (h,t)]

        # ================= phase 1: projections + rotary =================
        with ExitStack() as ph1:
            wqp = ph1.enter_context(tc.tile_pool(name="wqp", bufs=3))
            wkvp = ph1.enter_context(tc.tile_pool(name="wkvp", bufs=1))
            rotp = ph1.enter_context(tc.tile_pool(name="rotp", bufs=1))
            pqkv = ph1.enter_context(tc.tile_pool(name="pqkv", bufs=1, space="PSUM"))

            ps_xq = pqkv.tile([T, QW], f32, tag="ps_xq")
            ps_xk = pqkv.tile([T, HD], f32, tag="ps_xk")
            ps_xv = pqkv.tile([T, HD], f32, tag="ps_xv")

            wk_t = wkvp.tile([128, NDC * HD], pdt, tag="wk_t")
            nc.sync.dma_start(
                wk_t[:].rearrange("p (c d) -> p c d", c=NDC),
                wkT[:, :].rearrange("(c p) d -> p c d", p=128),
            )
            wv_t = wkvp.tile([128, NDC * HD], pdt, tag="wv_t")
            nc.sync.dma_start(
                wv_t[:].rearrange("p (c d) -> p c d", c=NDC),
                wvT[:, :].rearrange("(c p) d -> p c d", p=128),
            )

            CPW = 4  # chunks per wq tile
            for j in range(NDC // CPW):
                wq_t = wqp.tile([128, CPW * QW], pdt, tag="wq_t")
                nc.sync.dma_start(
                    wq_t[:].rearrange("p (c d) -> p c d", c=CPW),
                    wqT[j * CPW * 128:(j + 1) * CPW * 128, :].rearrange(
                        "(c p) d -> p c d", p=128
                    ),
                )
                for jj in range(CPW):
                    c = j * CPW + jj
                    lhs = xT_s[:, c * T:(c + 1) * T]
                    nc.tensor.matmul(
                        ps_xq[:, :],
                        lhs,
                        wq_t[:, jj * QW:(jj + 1) * QW],
                        start=(c == 0),
                        stop=(c == NDC - 1),
                    )
            for c in range(NDC):
                lhs = xT_s[:, c * T:(c + 1) * T]
                nc.tensor.matmul(
                    ps_xk[:, :], lhs, wk_t[:, c * HD:(c + 1) * HD],
                    start=(c == 0), stop=(c == NDC - 1),
                )
            for c in range(NDC):
                lhs = xT_s[:, c * T:(c + 1) * T]
                nc.tensor.matmul(
                    ps_xv[:, :], lhs, wv_t[:, c * HD:(c + 1) * HD],
                    start=(c == 0), stop=(c == NDC - 1),
                )

            # rotary on q (scale folded into tables), in natural layout
            q_cos = rotp.tile([T, QW], f32, tag="q_cos")
            nc.vector.tensor_mul(q_cos[:], ps_xq[:], cosq_s[:])
            q_swp = rotp.tile([T, QW], f32, tag="q_swp")
            nc.vector.tensor_copy(q_swp[:, 0:QW:2], ps_xq[:, 1:QW:2])
            nc.vector.tensor_copy(q_swp[:, 1:QW:2], ps_xq[:, 0:QW:2])
            q_sin = rotp.tile([T, QW], f32, tag="q_sin")
            nc.vector.tensor_mul(q_sin[:], q_swp[:], sinq_s[:])
            xq_rot = rotp.tile([T, QW], f32, tag="xq_rot")
            nc.vector.tensor_add(xq_rot[:], q_cos[:], q_sin[:])

            # rotary on k
            k_cos = rotp.tile([T, HD], f32, tag="k_cos")
            nc.vector.tensor_mul(k_cos[:], ps_xk[:], cosk_s[:])
            k_swp = rotp.tile([T, HD], f32, tag="k_swp")
            nc.vector.tensor_copy(k_swp[:, 0:HD:2], ps_xk[:, 1:HD:2])
            nc.vector.tensor_copy(k_swp[:, 1:HD:2], ps_xk[:, 0:HD:2])
            k_sin = rotp.tile([T, HD], f32, tag="k_sin")
            nc.vector.tensor_mul(k_sin[:], k_swp[:], sink_s[:])
            nc.vector.tensor_add(xk_rot[:], k_cos[:], k_sin[:])

            nc.scalar.copy(xv_s[:], ps_xv[:])

            # transpose q per head -> xqT [hd, (h,t)]
            for h in range(HQ):
                ps_t = ptr.tile([128, 128], f32, tag="ps_t")
                nc.tensor.transpose(
                    ps_t[:, :], xq_rot[:, h * HD:(h + 1) * HD], ident[:, :]
                )
                nc.scalar.copy(xqT_s[:, h * T:(h + 1) * T], ps_t[:])

        # ================= phase 2: attention over batches =================
        with ExitStack() as ph2:
            kvp = ph2.enter_context(tc.tile_pool(name="kvp", bufs=3))
            kbfp = ph2.enter_context(tc.tile_pool(name="kbfp", bufs=2))
            ktp = ph2.enter_context(tc.tile_pool(name="ktp", bufs=6))
            prp = ph2.enter_context(tc.tile_pool(name="prp", bufs=2))
            bcp = ph2.enter_context(tc.tile_pool(name="bcp", bufs=2))
            psc = ph2.enter_context(tc.tile_pool(name="psc", bufs=2, space="PSUM"))
            ppv = ph2.enter_context(tc.tile_pool(name="ppv", bufs=2, space="PSUM"))
            psm = ph2.enter_context(tc.tile_pool(name="psm", bufs=1, space="PSUM"))
            pbc = ph2.enter_context(tc.tile_pool(name="pbc", bufs=1, space="PSUM"))
            ones_col = persist.tile([128, 1], adt, tag="ones_col")
            nc.gpsimd.memset(ones_col[:], 1.0)

            NF = NLC - 1  # 31 full chunks from the old cache
            for b in range(B):
                # --- load shifted cache window (k on sync ring, v on scalar) ---
                # loads via SWDGE: 16-lane descriptor generation on its own
                # queue rows; the HWDGE rings only carry writes/weights
                k_main = kvp.tile([128, NF * HD], f32, tag="k_main")
                nc.gpsimd.dma_start(
                    k_main[:].rearrange("p (c d) -> p c d", c=NF),
                    ck[b, S:S + NF * 128, :].rearrange("(c p) d -> p c d", p=128),
                )
                v_main = kvp.tile([128, NF * HD], f32, tag="v_main")
                nc.gpsimd.dma_start(
                    v_main[:].rearrange("p (c d) -> p c d", c=NF),
                    cv[b, S:S + NF * 128, :].rearrange("(c p) d -> p c d", p=128),
                )
                # last chunk: 112 old rows + 16 new rows
                k31 = kvp.tile([128, HD], f32, tag="k31")
                nc.gpsimd.dma_start(k31[0:112, :], ck[b, S + NF * 128:L, :])
                nc.sync.dma_start(k31[112:128, :], xk_rot[b * S:(b + 1) * S, :])
                v31 = kvp.tile([128, HD], f32, tag="v31")
                nc.gpsimd.dma_start(v31[0:112, :], cv[b, S + NF * 128:L, :])
                nc.sync.dma_start(v31[112:128, :], xv_s[b * S:(b + 1) * S, :])

                # --- write updated cache back (HWDGE, k on SP / v on ACT) ---
                nc.sync.dma_start(
                    cko[b, 0:NF * 128, :].rearrange("(c p) d -> p c d", p=128),
                    k_main[:].rearrange("p (c d) -> p c d", c=NF),
                )
                nc.sync.dma_start(cko[b, NF * 128:L, :], k31[:, :])
                for wq4 in range(4):
                    c0 = wq4 * 8
                    c1 = min(NF, c0 + 8)
                    nc.scalar.dma_start(
                        cvo[b, c0 * 128:c1 * 128, :].rearrange(
                            "(c p) d -> p c d", p=128
                        ),
                        v_main[:, c0 * HD:c1 * HD].rearrange(
                            "p (c d) -> p c d", c=c1 - c0
                        ),
                    )
                nc.scalar.dma_start(cvo[b, NF * 128:L, :], v31[:, :])

                # bf16 copies of k (for transposes) and v (for PV matmuls)
                if BF16_ATTN:
                    k_bf = kbfp.tile([128, NLC * HD], adt, tag="k_bf")
                    nc.vector.tensor_copy(k_bf[:, 0:NF * HD], k_main[:])
                    nc.vector.tensor_copy(k_bf[:, NF * HD:NLC * HD], k31[:])
                    v_bf = kbfp.tile([128, NLC * HD], adt, tag="v_bf")
                    nc.vector.tensor_copy(v_bf[:, 0:NF * HD], v_main[:])
                    nc.vector.tensor_copy(v_bf[:, NF * HD:NLC * HD], v31[:])

                # --- attention ---
                rhs_q = xqT_s[:].rearrange("p (h t) -> p h t", h=HQ)[
                    :, :, b * S:(b + 1) * S
                ]
                ps_pv = ppv.tile([128, HS], f32, tag="ps_pv")
                ps_sum = psm.tile([1, HS], f32, tag="ps_sum")

                for g in range(NLC // GRP):
                    ps_sc = psc.tile([128, GRP * HS], f32, tag="ps_sc")
                    for q4 in range(GRP // 4):
                        ps_t = ptr.tile([128, 4 * HD], adt, tag="ps_t")
                        for jj4 in range(4):
                            jj = q4 * 4 + jj4
                            c = g * GRP + jj
                            if BF16_ATTN:
                                ktile = k_bf[:, c * HD:(c + 1) * HD]
                            else:
                                ktile = k31[:, :] if c == NF else k_main[:, c * HD:(c + 1) * HD]
                            nc.tensor.transpose(
                                ps_t[:, jj4 * HD:(jj4 + 1) * HD], ktile, ident_a[:, :]
                            )
                        kT_t = ktp.tile([128, 4 * HD], adt, tag="kT_t")
                        nc.vector.tensor_copy(kT_t[:], ps_t[:])
                        for jj4 in range(4):
                            jj = q4 * 4 + jj4
                            nc.tensor.matmul(
                                ps_sc[:, jj * HS:(jj + 1) * HS],
                                kT_t[:, jj4 * HD:(jj4 + 1) * HD],
                                rhs_q,
                                start=True,
                                stop=True,
                            )
                    probs = prp.tile([128, GRP * HS], adt, tag="probs")
                    if use_mask:
                        pm = prp.tile([128, GRP * HS], f32, tag="pm")
                        nc.vector.tensor_add(
                            pm[:], ps_sc[:],
                            maskT_s[:, g * GRP * HS:(g + 1) * GRP * HS],
                        )
                        nc.scalar.activation(
                            probs[:], pm[:], mybir.ActivationFunctionType.Exp,
                        )
                    else:
                        nc.scalar.activation(
                            probs[:], ps_sc[:], mybir.ActivationFunctionType.Exp,
                        )
                    for jj in range(GRP):
                        c = g * GRP + jj
                        if BF16_ATTN:
                            vtile = v_bf[:, c * HD:(c + 1) * HD]
                        else:
                            vtile = v31[:, :] if c == NF else v_main[:, c * HD:(c + 1) * HD]
                        pslice = probs[:, jj * HS:(jj + 1) * HS]
                        nc.tensor.matmul(
                            ps_pv[:, :], vtile, pslice,
                            start=(c == 0), stop=(c == NLC - 1),
                        )
                        nc.tensor.matmul(
                            ps_sum[:, :], ones_col[:], pslice,
                            start=(c == 0), stop=(c == NLC - 1),
                        )

                rec = bcp.tile([1, HS], f32, tag="rec")
                nc.vector.reciprocal(rec[:], ps_sum[:])
                ps_bc = pbc.tile([128, HS], f32, tag="ps_bc")
                nc.tensor.matmul(ps_bc[:, :], ones_row[:], rec[:], start=True, stop=True)
                bc_s = bcp.tile([128, HS], f32, tag="bc_s")
                nc.scalar.copy(bc_s[:], ps_bc[:])
                attn_dst = attnT[:].rearrange("p (h t) -> p h t", h=HQ)[
                    :, :, b * S:(b + 1) * S
                ]
                nc.vector.tensor_mul(attn_dst, ps_pv[:], bc_s[:])

        # ================= phase 3: output projection + RS =================
        with ExitStack() as ph3:
            wop = ph3.enter_context(tc.tile_pool(name="wop", bufs=4))
            outp = ph3.enter_context(tc.tile_pool(name="outp", bufs=1))
            pout = ph3.enter_context(tc.tile_pool(name="pout", bufs=2, space="PSUM"))

            wo_tiles = []
            for h in range(HQ):
                wo_t = wop.tile([128, D], pdt, tag="wo_t")
                eng = nc.sync if h % 2 == 0 else nc.scalar
                eng.dma_start(wo_t[:], woT[h * HD:(h + 1) * HD, :])
                wo_tiles.append(wo_t)

            out_s = outp.tile([T, D], f32, tag="out_s")
            NO = D // 512
            NRS = 2
            for half in range(NRS):
                for nn in range(NO // NRS):
                    n = half * (NO // NRS) + nn
                    ps_o = pout.tile([T, 512], f32, tag="ps_o")
                    for h in range(HQ):
                        nc.tensor.matmul(
                            ps_o[:, :],
                            attnT[:, h * T:(h + 1) * T],
                            wo_tiles[h][:, n * 512:(n + 1) * 512],
                            start=(h == 0),
                            stop=(h == HQ - 1),
                        )
                    if n % 2 == 0:
                        nc.vector.tensor_copy(out_s[:, n * 512:(n + 1) * 512], ps_o[:])
                    else:
                        nc.scalar.copy(out_s[:, n * 512:(n + 1) * 512], ps_o[:])
                DH = D // NRS
                nc.sync.dma_start(
                    rs_in[half][:, :], out_s[:, half * DH:(half + 1) * DH]
                )
                nc.gpsimd.collective_compute(
                    "ReduceScatter",
                    mybir.AluOpType.add,
                    ins=[rs_in[half][:, :]],
                    outs=[rs_out[half][:, :]],
                    replica_groups=[list(range(NCORES))],
                )
                nc.sync.dma_start(y[:, half * DH:(half + 1) * DH], rs_out[half][:, :])

    nc.compile()
    return nc


def _get_nc(use_mask):
    key = ("nc", use_mask)
    if key not in _CACHE:
        _CACHE[key] = _build(use_mask)
    return _CACHE[key]


def _prep_inputs(x, mask, freqs_cis, cache_k, cache_v, wq, wk, wv, wo):
    import ml_dtypes

    pnp = ml_dtypes.bfloat16 if BF16_PROJ else np.float32

    def f(a):
        return np.ascontiguousarray(a)

    def fp(a):
        return np.ascontiguousarray(a, dtype=pnp)

    x = np.asarray(x, np.float32)
    mask = np.asarray(mask, np.float32)
    fc = np.asarray(freqs_cis, np.float32)
    cache_k = np.asarray(cache_k, np.float32)
    cache_v = np.asarray(cache_v, np.float32)
    wq = np.asarray(wq, np.float32)
    wk = np.asarray(wk, np.float32)
    wv = np.asarray(wv, np.float32)
    wo = np.asarray(wo, np.float32)

    xT = fp(x.reshape(T, D).T)

    cos = fc[:, :, 0]
    sin = fc[:, :, 1]
    cos2 = np.repeat(cos, 2, axis=1)            # [S, HD]
    sin2 = np.empty((S, HD), np.float32)
    sin2[:, 0::2] = -sin
    sin2[:, 1::2] = sin
    isq = np.float32(1.0 / math.sqrt(HD))
    cosq = np.tile(np.tile(cos2, (B, 1)), (1, HQ)) * isq   # [T, QW]
    sinq = np.tile(np.tile(sin2, (B, 1)), (1, HQ)) * isq
    cosk = f(np.tile(cos2, (B, 1)))                        # [T, HD]
    sink = f(np.tile(sin2, (B, 1)))
    use_mask = bool(np.any(mask))
    maskT = f(np.tile(mask.T, (1, HQ))) if use_mask else None  # [L, HQ*S]

    wqT_full = wq.T          # [D, NH*HD]
    wkT_full = wk.T
    wvT_full = wv.T
    woT_full = wo.T          # [NH*HD, D]

    in_maps = []
    for i in range(NCORES):
        in_maps.append({
            "xT": xT,
            "wqT": fp(wqT_full[:, i * QW:(i + 1) * QW]),
            "wkT": fp(wkT_full[:, i * HD:(i + 1) * HD]),
            "wvT": fp(wvT_full[:, i * HD:(i + 1) * HD]),
            "woT": fp(woT_full[i * QW:(i + 1) * QW, :]),
            "ck": f(cache_k[:, :, i, :]),
            "cv": f(cache_v[:, :, i, :]),
            "cosq": f(cosq),
            "sinq": f(sinq),
            "cosk": cosk,
            "sink": sink,
        })
        if use_mask:
            in_maps[-1]["maskT"] = maskT
    return in_maps, use_mask


def kernel(x, mask, freqs_cis, cache_k, cache_v, wq, wk, wv, wo):
    from concourse.bass_utils import run_bass_kernel_spmd

    in_maps, use_mask = _prep_inputs(
        x, mask, freqs_cis, cache_k, cache_v, wq, wk, wv, wo
    )
    nc = _get_nc(use_mask)
    res = run_bass_kernel_spmd(nc, in_maps, core_ids=list(range(NCORES)))
    outs = res.results

    out = np.empty((B, S, D), np.float32)
    for i in range(NCORES):
        out[i] = outs[i]["y"]
    ck_new = np.stack([outs[i]["cko"] for i in range(NCORES)], axis=2)
    cv_new = np.stack([outs[i]["cvo"] for i in range(NCORES)], axis=2)
    return out, ck_new, cv_new
